# revision 1
# baseline (speedup 1.0000x reference)
"""Trainium2 Bass kernel for nn_DentalAnatomyLoss.

Computes, for segmentation [B=2, C=32, D=64, H=128, W=128] fp32:
  - crown/root ratio loss (per (b,c) sums over d<32 / d>=32)
  - 3D total-variation loss (mean |diff| along w, h, d)
  - returns stack([crown_root, smoothness, total_anatomy]) fp32 [3]

Strategy: pure data-parallel over the 64 (b,c) slices, 8 per NeuronCore.
Each core reduces its 32 MiB shard to a [128, 160] fp32 partial tensor;
the host combines partials into the 3 scalars.

Per-core engine split (memory regime, ~94 us HBM roofline/core):
  - ScalarE: fp32->bf16 cast with fused accum_out (crown/root sums), and
    Abs+accum_out consuming the h-diff matmul output from PSUM.
  - VectorE: the w-diff as one fused scalar_tensor_tensor (out=max(a,b),
    accum_out=sum) reading fp32 directly (the shift-by-one AP is 1x in
    any dtype); the d-diff as an aligned 2x subtract + 4x fused relu-sum.
    The host recovers sum|a-b| = 2*sum(max(a,b)) - sum(a) - sum(b) and
    sum|d| = 2*sum(max(d,0)) - sum(d), with the signed sums telescoping
    to boundary-column sums.
  - TensorE: bidiagonal matmul computes h-diffs (partition axis) in PSUM.
  - DMA: HBM loads only (the SP ring), ~94 us/core at ~360 GB/s.

Pipelining: xb-dependent work (d-diff, h-diff matmul) is emitted one
chunk late so VectorE never waits on the cast; PSUM is two half-chunk
tiles (4 banks each) so TensorE fills one while ScalarE drains the
other; each PSUM drain is deferred past the next fill.
"""

import os

import numpy as np

B, C, D, H, W = 2, 32, 64, 128, 128
NCORES = 8
JPC = (B * C) // NCORES  # (b,c) slices per core
CROWN_ROOT_W = 2.0
SMOOTH_W = 1.5
EXPECTED_RATIO = 1.2

# accumulator column layout in the [128, ACC_COLS] partial tensor
# (one column per chunk = (slice j, half); 16 chunks per core)
ACC_COLS = 160
COL_X = 0  # 16: sum(x) per chunk
COL_DXP = 16  # 16: sum(max(x[...,w], x[...,w+1])) over w-pairs
COL_TXF = 32  # 16: sum over planes of column w=0
COL_TXL = 48  # 16: sum over planes of column w=W-1
COL_DZP = 64  # 16: sum(max(dz,0)), dz = plane[k+1]-plane[k] (in-chunk)
COL_TZF = 80  # 16: sum of first plane of chunk
COL_TZL = 96  # 16: sum of last plane of chunk
COL_DY = 112  # 32: sum|dy| per (chunk, psum-half)
COL_BNDP = 144  # 8: sum(max(a,b)) for the half0/half1 boundary plane pair
# 152:160 unused (zeroed)

_PROG_CACHE: dict = {}
last_exec_time_ns = None  # set by kernel() when tracing is enabled


def _build_program(jpc=JPC, d=D, h=H, w=W, repeat=1, skip=()):
    """Build the (single) SPMD Bass program run identically on all cores.

    repeat>1 wraps the whole compute in a hardware For_i loop (identical
    result, used only for wall-clock timing of the kernel body).
    """
    from contextlib import ExitStack

    import concourse.tile as tile
    from concourse import bacc, mybir

    f32 = mybir.dt.float32
    bf16 = mybir.dt.bfloat16
    AO = mybir.AluOpType
    AF = mybir.ActivationFunctionType

    ndh = d // 2  # planes per chunk; chunks never straddle the crown/root split
    fsz = ndh * w  # free size of one chunk

    nc = bacc.Bacc(
        "TRN2",
        target_bir_lowering=False,
        debug=False,
        enable_asserts=False,
        num_devices=NCORES,
    )
    seg = nc.dram_tensor("seg", [jpc, d, h, w], f32, kind="ExternalInput").ap()
    bd = nc.dram_tensor("bidiag", [h, h], bf16, kind="ExternalInput").ap()
    out = nc.dram_tensor("partials", [h, ACC_COLS], f32, kind="ExternalOutput").ap()

    with tile.TileContext(nc) as tc, ExitStack() as ctx:
        singles = ctx.enter_context(tc.tile_pool(name="singles", bufs=1))
        x32p = ctx.enter_context(tc.tile_pool(name="x32", bufs=3))
        xbp = ctx.enter_context(tc.tile_pool(name="xb", bufs=4))
        dxp = ctx.enter_context(tc.tile_pool(name="dx", bufs=2))
        dzp = ctx.enter_context(tc.tile_pool(name="dz", bufs=2))
        tinyp = ctx.enter_context(tc.tile_pool(name="tiny", bufs=2))
        dummyp = ctx.enter_context(tc.tile_pool(name="dummy", bufs=4))
        psp = ctx.enter_context(tc.tile_pool(name="ps", bufs=2, space="PSUM"))

        bd_sb = singles.tile([h, h], bf16)
        nc.sync.dma_start(out=bd_sb, in_=bd)
        acc = singles.tile([h, ACC_COLS], f32)
        nc.vector.memset(acc, 0.0)

        nblk = fsz // 512  # matmul free-dim blocks (512 = one PSUM bank)
        planes_per_blk = 512 // w
        nsub = 2 if nblk % 2 == 0 and nblk >= 2 else 1
        hb = nblk // nsub  # psum blocks per half-chunk tile

        def sum_max(out_ap, a_ap, b_ap, col):
            """out = max(a,b); acc[:,col] = sum(out). out is write-only."""
            nc.vector.scalar_tensor_tensor(
                out=out_ap,
                in0=a_ap,
                scalar=0.0,
                in1=b_ap,
                op0=AO.bypass,
                op1=AO.max,
                accum_out=acc[:, col : col + 1],
            )

        def sum_relu(src_ap, col):
            """acc[:,col] = sum(max(src,0)); src rewritten in place."""
            nc.vector.tensor_scalar(
                out=src_ap,
                in0=src_ap,
                scalar1=0.0,
                scalar2=None,
                op0=AO.max,
                op1=AO.add,
                accum_out=acc[:, col : col + 1],
            )

        def sum_ident(src_ap, col):
            """acc[:,col] = sum(src); src rewritten in place (x + 0.0).

            Only used on tiles of non-negative values (x in [0,1)), so the
            identity rewrite is bit-exact.
            """
            nc.vector.tensor_scalar(
                out=src_ap,
                in0=src_ap,
                scalar1=0.0,
                scalar2=None,
                op0=AO.add,
                op1=AO.add,
                accum_out=acc[:, col : col + 1],
            )

        state = {"prev_xb": None, "pending_gy": None, "pending_c": None}

        def emit_gy(ps_tile, cidx, sub):
            dya = dummyp.tile([h, 1], bf16)
            col = COL_DY + nsub * cidx + sub
            nc.scalar.activation(
                out=dya.broadcast_to((h, hb, 512)),
                in_=ps_tile[:, :, :],
                func=AF.Abs,
                accum_out=acc[:, col : col + 1],
            )

        def stage_c(j, half, cidx, xb, xbf):
            """xb-dependent work, emitted one chunk late (see module doc)."""
            # h-diff (gy) via bidiagonal matmul into PSUM; two half-chunk
            # tiles so PE fills one while ScalarE drains the other, and each
            # drain is deferred past the next fill.
            if "gy" not in skip:
                for sub in range(nsub):
                    ps = psp.tile([h, hb, 512], f32)
                    for blk in range(hb):
                        g = sub * hb + blk
                        nc.tensor.matmul(
                            ps[:, blk, :],
                            bd_sb,
                            xb[:, g * planes_per_blk : (g + 1) * planes_per_blk, :],
                            start=True,
                            stop=True,
                        )
                    if state["pending_gy"] is not None:
                        emit_gy(*state["pending_gy"])
                    state["pending_gy"] = (ps, cidx, sub)

            # d-diff (gz), in-chunk pairs: aligned TT subtract (2x) then
            # fused relu-sum (4x); sum(dz) telescopes on host.
            if "dz" not in skip:
                dz = dzp.tile([h, fsz - w], bf16)
                nc.vector.tensor_tensor(
                    out=dz,
                    in0=xbf[:, w:fsz],
                    in1=xbf[:, 0 : fsz - w],
                    op=AO.subtract,
                )
                sum_relu(dz[:, :], COL_DZP + cidx)
                # first/last plane sums for the signed sums
                sum_ident(xb[:, 0, :], COL_TZF + cidx)
                sum_ident(xb[:, ndh - 1, :], COL_TZL + cidx)

                # boundary pair between the two halves of slice j
                if half == 1:
                    bnd = tinyp.tile([h, w], bf16)
                    sum_max(
                        bnd,
                        xb[:, 0, :],
                        state["prev_xb"][:, ndh - 1, :],
                        COL_BNDP + j,
                    )
                state["prev_xb"] = xb

        def chunk_body(j, half):
            cidx = j * 2 + half
            d0 = half * ndh

            # 1) load chunk: [h partitions, ndh planes, w] fp32
            x32 = x32p.tile([h, ndh, w], f32)
            nc.sync.dma_start(
                out=x32, in_=seg[j, d0 : d0 + ndh, :, :].rearrange("d h w -> h d w")
            )

            # 2) cast to bf16; fused accum -> crown/root sum for this chunk
            if "conv" in skip:
                return
            xb = xbp.tile([h, ndh, w], bf16)
            nc.scalar.activation(
                out=xb,
                in_=x32,
                func=AF.Copy,
                accum_out=acc[:, COL_X + cidx : COL_X + cidx + 1],
            )
            xbf = xb.rearrange("p a b -> p (a b)")

            # 3) w-diff (gx): one fused op per chunk.  The exact 3D AP
            #    (misaligned by one element) runs at 1x either way, so it
            #    reads the fp32 tile directly: no dependency on the cast,
            #    and full fp32 precision for the gx term.
            # 4) run the previous chunk's deferred xb-dependent work FIRST:
            #    it is ready now, while this chunk's dx still waits on its
            #    DMA -- this order lets VectorE cover DMA latency
            if state["pending_c"] is not None:
                stage_c(*state["pending_c"])
            state["pending_c"] = (j, half, cidx, xb, xbf)

            if "dx" not in skip:
                dx = dxp.tile([h, ndh, w - 1], bf16)
                sum_max(dx, x32[:, :, 1:], x32[:, :, 0 : w - 1], COL_DXP + cidx)
                # boundary-column sums for the signed sums (fp32)
                sum_ident(x32[:, :, 0:1], COL_TXF + cidx)
                sum_ident(x32[:, :, w - 1 : w], COL_TXL + cidx)

        def all_chunks():
            for j in range(jpc):
                for half in range(2):
                    chunk_body(j, half)
            if state["pending_c"] is not None:
                stage_c(*state["pending_c"])
            state["pending_c"] = None
            if state["pending_gy"] is not None:
                emit_gy(*state["pending_gy"])
            state["pending_gy"] = None

        if repeat == 1:
            all_chunks()
        else:
            with tc.For_i(0, repeat, 1):
                all_chunks()
        nc.sync.dma_start(out=out, in_=acc)

    nc.compile()
    return nc


def _get_program():
    key = "full"
    if key not in _PROG_CACHE:
        _PROG_CACHE[key] = _build_program()
    return _PROG_CACHE[key]


def _bidiag_np(h=H):
    """lhsT for the h-diff matmul: out[m,:] = rhs[m+1,:] - rhs[m,:]."""
    import ml_dtypes

    m = np.zeros((h, h), dtype=np.float32)
    for c in range(h - 1):
        m[c + 1, c] = 1.0
        m[c, c] = -1.0
    # last column stays zero -> output row h-1 is 0
    return m.astype(ml_dtypes.bfloat16)


def _combine(partials, b=B, c=C, d=D, h=H, w=W):
    """Host-side finish: per-core [128, 160] fp32 partials -> [3] fp32."""
    nslice = b * c
    jpc = nslice // len(partials)

    crown = np.zeros(nslice, dtype=np.float64)
    root = np.zeros(nslice, dtype=np.float64)
    gx_sum = 0.0
    gy_sum = 0.0
    gz_sum = 0.0
    for k, p in enumerate(partials):
        p = p.astype(np.float64)
        xp = p[:, COL_DXP : COL_DXP + 2 * jpc].sum(axis=0)
        txf = p[:, COL_TXF : COL_TXF + 2 * jpc].sum(axis=0)
        txl = p[:, COL_TXL : COL_TXL + 2 * jpc].sum(axis=0)
        zp = p[:, COL_DZP : COL_DZP + 2 * jpc].sum(axis=0)
        tzf = p[:, COL_TZF : COL_TZF + 2 * jpc].sum(axis=0)
        tzl = p[:, COL_TZL : COL_TZL + 2 * jpc].sum(axis=0)
        bndp = p[:, COL_BNDP : COL_BNDP + jpc].sum(axis=0)

        xs = p[:, COL_X : COL_X + 2 * jpc].sum(axis=0)
        # sum|a-b| = 2*sum(max(a,b)) - sum(a) - sum(b)
        # gx: a = x[..., 1:], b = x[..., :-1]
        gx_sum += (2.0 * xp - (xs - txf) - (xs - txl)).sum()
        # gz: dz = planes[1:] - planes[:-1]; sum(dz) = tzl - tzf
        gz_sum += (2.0 * zp - (tzl - tzf)).sum()
        # boundary pair: a = half1.plane0, b = half0.plane(ndh-1)
        for jj in range(jpc):
            gz_sum += 2.0 * bndp[jj] - tzf[2 * jj + 1] - tzl[2 * jj]
        gy_sum += p[:, COL_DY : COL_DY + 4 * jpc].sum()

        for jj in range(jpc):
            crown[k * jpc + jj] = p[:, COL_X + 2 * jj].sum()
            root[k * jpc + jj] = p[:, COL_X + 2 * jj + 1].sum()

    total = crown + root
    valid = (total > 0) & (root > 0)
    safe_root = np.where(root > 0, root, 1.0)
    ratio_loss = np.where(valid, (crown / safe_root - EXPECTED_RATIO) ** 2, 0.0)
    cr_loss = ratio_loss.sum() / nslice

    nx = nslice * d * h * (w - 1)
    ny = nslice * d * (h - 1) * w
    nz = nslice * (d - 1) * h * w
    tv = gx_sum / nx + gy_sum / ny + gz_sum / nz

    crown_root = cr_loss * CROWN_ROOT_W
    smoothness = tv * SMOOTH_W
    return np.array(
        [crown_root, smoothness, crown_root + smoothness], dtype=np.float32
    )


def kernel(segmentation: np.ndarray) -> np.ndarray:
    global last_exec_time_ns
    from concourse.bass_utils import run_bass_kernel_spmd

    seg = np.ascontiguousarray(np.asarray(segmentation), dtype=np.float32)
    assert seg.shape == (B, C, D, H, W)
    nc = _get_program()

    bd = _bidiag_np()
    shards = seg.reshape(B * C, D, H, W)
    in_maps = [
        {"seg": np.ascontiguousarray(shards[k * JPC : (k + 1) * JPC]), "bidiag": bd}
        for k in range(NCORES)
    ]
    trace = bool(os.environ.get("BASS_TRACE"))
    res = run_bass_kernel_spmd(nc, in_maps, list(range(NCORES)), trace=trace)
    last_exec_time_ns = res.exec_time_ns
    partials = [res.results[k]["partials"] for k in range(NCORES)]
    return _combine(partials)



# revision 2
# speedup vs baseline: 3.9633x; 3.9633x over previous
"""Trainium2 Bass kernel for nn_DentalAnatomyLoss.

Computes, for segmentation [B=2, C=32, D=64, H=128, W=128] fp32:
  - crown/root ratio loss (per (b,c) sums over d<32 / d>=32)
  - 3D total-variation loss (mean |diff| along w, h, d)
  - returns stack([crown_root, smoothness, total_anatomy]) fp32 [3]

Strategy: pure data-parallel over the 64 (b,c) slices, 8 per NeuronCore.
Each core reduces its 32 MiB shard to a [128, 52] fp32 partial tensor;
the host combines partials into the 3 scalars.

Layout: d-on-partitions. Each "chunk pair" (cp) holds 2 slices:
partition p = s*64 + d for local slice s in {0,1}, plane d in 0..63;
free axis = (h, w) = 16384 bf16. Benefits over the h-partition layout:
  - DMA reads are 16 KiB contiguous per partition (vs 512 B rows), and
    the fp32->bf16 cast happens inside the SWDGE DMA (measured at full
    HBM rate), freeing ScalarE entirely from casting.
  - The h-diff (gy) becomes an aligned free-axis shift by w -> one fused
    scalar_tensor_tensor max+accum per cp on VectorE.
  - The d-diff (gz) is the partition-axis diff -> TensorE block-bidiag
    matmul into PSUM (columns 63/127 zeroed so no cross-slice pairs),
    drained by ScalarE Abs+accum. Rows 63/127 drain |0| = 0.

Per-core engine budget (measured sustained rates):
  VectorE ~136 us: gx + gy fused STT max+accum (1x; sweeping 2x modes
    does not help: any elementwise+reduce pair costs the same 2 touches).
  ScalarE ~131 us: per-plane sum(x) via broadcast-out Copy+accum (fp32
    exact, feeds crown/root and the max-trick telescopes), PSUM drains,
    and the tiny first/last row/col telescope sums.
  TensorE ~70 us, DMA ~100 us (HBM roofline ~94 us/core).

Host recovers sum|a-b| = 2*sum(max(a,b)) - sum(a) - sum(b); the signed
sums telescope to per-plane sums and first/last row/col sums. gx and gy
share one denominator (d*h*(w-1) == d*(h-1)*w), gz has its own.
"""

import os

import numpy as np

B, C, D, H, W = 2, 32, 64, 128, 128
NCORES = 8
JPC = (B * C) // NCORES  # slices per core
CROWN_ROOT_W = 2.0
SMOOTH_W = 1.5
EXPECTED_RATIO = 1.2

# accumulator column layout in the [128, ACC_COLS] partial tensor
NCP = JPC // 2  # chunk pairs per core
COL_SX = 0  # NCP: per-plane sum(x)
COL_GY = COL_SX + NCP  # NCP: per-plane sum(max(x[h+1], x[h]))
COL_GX = COL_GY + NCP  # NCP: per-plane sum(max(x[w+1], x[w]))
COL_R = COL_GX + NCP  # NCP: per-plane sum(row0 + row127)
COL_C = COL_R + NCP  # NCP: per-plane sum(col0 + col127)
COL_DZ = COL_C + NCP  # NCP*NDRAIN: PSUM |dz| drains
NDRAIN = 8
ACC_COLS = COL_DZ + NCP * NDRAIN

_PROG_CACHE: dict = {}
last_exec_time_ns = None


def _build_program(jpc=JPC, d=D, h=H, w=W, repeat=1, skip=()):
    """Build the (single) SPMD Bass program run identically on all cores.

    repeat>1 wraps the whole compute in a hardware For_i loop (identical
    result, used only for wall-clock timing of the kernel body).
    """
    from contextlib import ExitStack

    import concourse.tile as tile
    from concourse import bacc, mybir

    f32 = mybir.dt.float32
    bf16 = mybir.dt.bfloat16
    AO = mybir.AluOpType
    AF = mybir.ActivationFunctionType

    ncp = jpc // 2
    P = 2 * d  # partitions per chunk pair
    fsz = h * w  # free size per partition (one (h,w) plane)
    nq = 4  # DMA splits per chunk pair
    qsz = fsz // nq
    nblk = fsz // 512  # 512-col matmul blocks per cp
    ndrain = NDRAIN if fsz == 16384 else max(1, nblk // 4)
    blk_per_drain = nblk // ndrain
    dsz = blk_per_drain * 512  # free size of one PSUM drain tile

    acc_cols = COL_C + ncp + ncp * ndrain
    col_dz = COL_C + ncp

    nc = bacc.Bacc(
        "TRN2",
        target_bir_lowering=False,
        debug=False,
        enable_asserts=False,
        num_devices=NCORES,
    )
    seg = nc.dram_tensor("seg", [jpc, d, h, w], f32, kind="ExternalInput").ap()
    bd = nc.dram_tensor("bidiag", [P, P], bf16, kind="ExternalInput").ap()
    out = nc.dram_tensor("partials", [P, acc_cols], f32, kind="ExternalOutput").ap()

    with tile.TileContext(nc) as tc, ExitStack() as ctx:
        singles = ctx.enter_context(tc.tile_pool(name="singles", bufs=1))
        xbp = ctx.enter_context(tc.tile_pool(name="xb", bufs=3))
        scrp = ctx.enter_context(tc.tile_pool(name="scr", bufs=2))
        dumbp = ctx.enter_context(tc.tile_pool(name="dumb", bufs=2))
        psp = ctx.enter_context(tc.tile_pool(name="ps", bufs=2, space="PSUM"))

        bd_sb = singles.tile([P, P], bf16)
        nc.sync.dma_start(out=bd_sb, in_=bd)
        acc = singles.tile([P, acc_cols], f32)
        nc.vector.memset(acc, 0.0)

        def cp_body(c):
            # 1) SWDGE cast-DMA loads: fp32 HBM -> bf16 SBUF, d-layout.
            #    Per partition: contiguous 4*qsz bytes from DRAM.
            xb = xbp.tile([P, fsz], bf16)
            src = seg[2 * c : 2 * c + 2].rearrange("s d h w -> (s d) (h w)")
            for q in range(nq):
                nc.gpsimd.dma_start(
                    out=xb[:, q * qsz : (q + 1) * qsz],
                    in_=src[:, q * qsz : (q + 1) * qsz],
                )

            scratch = scrp.tile([P, fsz - w], bf16)
            dummy = dumbp.tile([P, 1], bf16)

            # 2) VectorE: fused max+accum for gy (aligned shift by w) and
            #    gx (shift by 1 inside each w-row). Both 1x; one op each.
            if "gy" not in skip:
                nc.vector.scalar_tensor_tensor(
                    out=scratch,
                    in0=xb[:, w:fsz],
                    scalar=0.0,
                    in1=xb[:, 0 : fsz - w],
                    op0=AO.bypass,
                    op1=AO.max,
                    accum_out=acc[:, COL_GY + c : COL_GY + c + 1],
                )
            if "gx" not in skip:
                xb3 = xb.rearrange("p (r c2) -> p r c2", c2=w)
                scr3 = scratch.rearrange("p (r c2) -> p r c2", c2=w - 1)[
                    :, 0:h, :
                ]
                nc.vector.scalar_tensor_tensor(
                    out=scr3,
                    in0=xb3[:, :, 1:w],
                    scalar=0.0,
                    in1=xb3[:, :, 0 : w - 1],
                    op0=AO.bypass,
                    op1=AO.max,
                    accum_out=acc[:, COL_GX + c : COL_GX + c + 1],
                )

            # 3) ScalarE: per-plane sum(x) (exact fp32 accum); telescope
            #    row/col sums; all via broadcast-out Copy+accum.
            if "sx" not in skip:
                nc.scalar.activation(
                    out=dummy.broadcast_to((P, fsz)),
                    in_=xb,
                    func=AF.Copy,
                    accum_out=acc[:, COL_SX + c : COL_SX + c + 1],
                )
                # sum(row0 + row_{h-1}) per plane
                rows = xb.rearrange("p (r c2) -> p r c2", c2=w)[
                    :, 0 : h : h - 1, :
                ]
                nc.scalar.activation(
                    out=dummy.broadcast_to((P, 2, w)),
                    in_=rows,
                    func=AF.Copy,
                    accum_out=acc[:, COL_R + c : COL_R + c + 1],
                )
                # sum(col0 + col_{w-1}) per plane
                cols = xb.rearrange("p (r c2) -> p c2 r", c2=w)[
                    :, 0 : w : w - 1, :
                ]
                nc.scalar.activation(
                    out=dummy.broadcast_to((P, 2, h)),
                    in_=cols,
                    func=AF.Copy,
                    accum_out=acc[:, COL_C + c : COL_C + c + 1],
                )

            # 4) TensorE block-bidiag d-diffs -> PSUM; ScalarE Abs drains.
            if "gz" not in skip:
                for t in range(ndrain):
                    ps = psp.tile([P, blk_per_drain, 512], f32)
                    for b in range(blk_per_drain):
                        blk = t * blk_per_drain + b
                        nc.tensor.matmul(
                            ps[:, b, :],
                            bd_sb,
                            xb[:, blk * 512 : (blk + 1) * 512],
                            start=True,
                            stop=True,
                        )
                    col = col_dz + ndrain * c + t
                    nc.scalar.activation(
                        out=dummy.broadcast_to((P, blk_per_drain, 512)),
                        in_=ps[:, :, :],
                        func=AF.Abs,
                        accum_out=acc[:, col : col + 1],
                    )

        def all_cps():
            for c in range(ncp):
                cp_body(c)

        if repeat == 1:
            all_cps()
        else:
            with tc.For_i(0, repeat, 1):
                all_cps()
        nc.sync.dma_start(out=out, in_=acc)

    nc.compile()
    return nc


def _get_program():
    key = "full"
    if key not in _PROG_CACHE:
        _PROG_CACHE[key] = _build_program()
    return _PROG_CACHE[key]


def _bidiag_np(d=D):
    """lhsT for the d-diff matmul: out[m,:] = x[m+1,:] - x[m,:] within
    each slice; columns d-1 and 2d-1 zeroed (no cross-slice pairs)."""
    import ml_dtypes

    P = 2 * d
    m = np.zeros((P, P), dtype=np.float32)
    for col in range(P - 1):
        if col == d - 1:
            continue
        m[col, col] = -1.0
        m[col + 1, col] = 1.0
    return m.astype(ml_dtypes.bfloat16)


def _combine(partials, jpc=JPC, d=D, h=H, w=W):
    """Host-side finish: per-core [2d, acc_cols] fp32 partials -> [3]."""
    ncp = jpc // 2
    fsz = h * w
    nblk = fsz // 512
    ndrain = NDRAIN if fsz == 16384 else max(1, nblk // 4)
    col_dz = COL_C + ncp

    nslice = jpc * len(partials)
    crown = np.zeros(nslice, dtype=np.float64)
    root = np.zeros(nslice, dtype=np.float64)
    gxy_sum = 0.0
    gz_sum = 0.0
    for k, p in enumerate(partials):
        p = p.astype(np.float64)
        for c in range(ncp):
            sx = p[:, COL_SX + c]  # per-plane sum(x)
            gy = p[:, COL_GY + c]  # per-plane sum(max over h-pairs)
            gx = p[:, COL_GX + c]  # per-plane sum(max over w-pairs)
            rr = p[:, COL_R + c]  # per-plane sum(row0 + row_{h-1})
            cc = p[:, COL_C + c]  # per-plane sum(col0 + col_{w-1})
            # sum|a-b| = 2*sum(max) - sum(a) - sum(b); the signed sums
            # telescope: gy: -2*sx + rr ; gx: -2*sx + cc (per plane).
            gxy_sum += (2.0 * gy - 2.0 * sx + rr).sum()
            gxy_sum += (2.0 * gx - 2.0 * sx + cc).sum()
            for s in (0, 1):
                sl = k * jpc + 2 * c + s
                crown[sl] = sx[s * d : s * d + d // 2].sum()
                root[sl] = sx[s * d + d // 2 : s * d + d].sum()
        dz = p[:, col_dz : col_dz + ncp * ndrain]
        # rows d-1 and 2d-1 are |0| = 0 (zeroed bidiag columns)
        gz_sum += dz.sum()

    total = crown + root
    valid = (total > 0) & (root > 0)
    safe_root = np.where(root > 0, root, 1.0)
    ratio_loss = np.where(valid, (crown / safe_root - EXPECTED_RATIO) ** 2, 0.0)
    cr_loss = ratio_loss.sum() / nslice

    nxy = nslice * d * h * (w - 1)  # == nslice * d * (h-1) * w
    nz = nslice * (d - 1) * h * w
    tv = gxy_sum / nxy + gz_sum / nz

    crown_root = cr_loss * CROWN_ROOT_W
    smoothness = tv * SMOOTH_W
    return np.array(
        [crown_root, smoothness, crown_root + smoothness], dtype=np.float32
    )


def kernel(segmentation: np.ndarray) -> np.ndarray:
    global last_exec_time_ns
    from concourse.bass_utils import run_bass_kernel_spmd

    seg = np.ascontiguousarray(np.asarray(segmentation), dtype=np.float32)
    assert seg.shape == (B, C, D, H, W)
    nc = _get_program()

    bd = _bidiag_np()
    shards = seg.reshape(B * C, D, H, W)
    in_maps = [
        {"seg": np.ascontiguousarray(shards[k * JPC : (k + 1) * JPC]), "bidiag": bd}
        for k in range(NCORES)
    ]
    trace = bool(os.environ.get("BASS_TRACE"))
    res = run_bass_kernel_spmd(nc, in_maps, list(range(NCORES)), trace=trace)
    last_exec_time_ns = res.exec_time_ns
    partials = [res.results[k]["partials"] for k in range(NCORES)]
    return _combine(partials)
